# revision 2
# baseline (speedup 1.0000x reference)
"""Trainium2 Bass kernel for the GNN message-passing draft problem.

Math notes (exact simplifications of the reference):
- softmax over key nodes j makes scores' sq/bqk terms cancel
  (shift invariance), so w[i,j,b] = softmax_j(sk[j,b]) independent of i.
- Therefore after round 1 the node state is constant across nodes, and
  rounds 2/3 collapse to per-batch MLPs:  x <- relu((x@Wv+bv)@Wa+ba).
- Round 1 aggregation commutes with Wv:  aggre = (sum_j w[j,b] x_j) @ Wv + bv.
- (As@W_emb + b_emb)@W_h + b_h == As@(W_emb@W_h) + (b_emb@W_h + b_h).
- Wq, bq, bk, bqk never affect the output.

Per core (8 cores, data-parallel over batch): As shard [N=128 nodes,
B_loc=128, F=512] flattened to rows (j,b) j-major = [16384, 512].
Stage 1 streams As, transposes 128x128 blocks on PE, and runs matmuls
against the folded weight to produce xT [h=128, cols=(j,b)].

Performance structure (vs the earlier baseline):
- sk is computed with a replicated-u stationary so one matmul yields sk
  broadcast across all 128 partitions; exp of that directly gives the
  broadcast attention weights (no separate ones-outer-product matmul).
- softmax denominator accumulated as one [1,512] add per step instead of
  4 gpsimd adds.
- transposes run in f32r (1.5 cyc/row) or bf16 (1 cyc/row, via SWDGE
  cast-DMA loads); evictions are spread across DVE/ACT/Pool.
- As is loaded in 2 MiB transfers (8 node-tiles), 2 compute steps each.
"""

import sys

sys.path.insert(0, "/opt/trn_rl_repo")

from contextlib import ExitStack

import numpy as np

import concourse.bass as bass
import concourse.tile as tile
from concourse import bacc, mybir
from concourse.bass_utils import run_bass_kernel_spmd

F32 = mybir.dt.float32
F32R = mybir.dt.float32r
BF16 = mybir.dt.bfloat16
AF = mybir.ActivationFunctionType
ALU = mybir.AluOpType

N_NODES, BATCH, FEAT, EMB, HID = 128, 1024, 512, 256, 128
NCORES = 8
BLOC = BATCH // NCORES          # 128 batch elements per core
ROWS = N_NODES * BLOC           # 16384 rows per core
P = 128
TPS = 4                         # node-tiles per compute step
TPL = 8                         # node-tiles per DMA transfer (2 MiB f32)
NSTEPS = N_NODES // TPS         # 32 compute steps
NLOADS = N_NODES // TPL         # 16 DMA transfers
CW = TPS * P                    # 512 columns per compute step


def build(repeat=1, upto="full", variant="bf16", evict_eng="vvvv",
          s_eng="p", load_bufs=4):
    """variant: 'bf16' = SWDGE cast-DMA loads + bf16 transposes;
    'f32r' = plain HWDGE f32 loads + f32r transposes."""
    nc = bacc.Bacc(None, target_bir_lowering=False, debug=False)
    v3 = variant == "bf16"

    dI = lambda name, shape: nc.dram_tensor(name, shape, F32, kind="ExternalInput").ap()
    As_d = dI("As", [ROWS, FEAT])
    W_emb_d = dI("W_emb", [FEAT, EMB])
    b_emb_d = dI("b_emb", [EMB])
    W_h_d = dI("W_h", [EMB, HID])
    b_h_d = dI("b_h", [HID])
    Wk_d = dI("Wk", [HID, HID])
    Wqk_d = dI("Wqk", [2 * HID, 1])
    Wv_d = dI("Wv", [HID, HID])
    bv_d = dI("bv", [HID])
    Wa_d = dI("Wa", [HID, HID])
    ba_d = dI("ba", [HID])
    W1_d = dI("W1", [HID, HID])
    b1_d = dI("b1", [HID])
    W2_d = dI("W2", [HID, FEAT])
    b2_d = dI("b2", [FEAT])
    eye_d = dI("eye", [P, P])
    out_d = nc.dram_tensor("out", [BLOC, FEAT], F32, kind="ExternalOutput").ap()

    with tile.TileContext(nc) as tc, ExitStack() as ctx:
        const = ctx.enter_context(tc.tile_pool(name="const", bufs=1))
        work = ctx.enter_context(tc.tile_pool(name="work", bufs=3))
        load = ctx.enter_context(tc.tile_pool(name="load", bufs=load_bufs))
        astp = ctx.enter_context(tc.tile_pool(name="astp", bufs=3))
        xsl = ctx.enter_context(tc.tile_pool(name="xsl", bufs=3))
        tp_ps = ctx.enter_context(tc.tile_pool(name="tp_ps", bufs=3, space="PSUM"))
        x_ps = ctx.enter_context(tc.tile_pool(name="x_ps", bufs=2, space="PSUM"))
        sk_ps = ctx.enter_context(tc.tile_pool(name="sk_ps", bufs=2, space="PSUM"))

        # ---------------- constants / weights ----------------
        ident_f = const.tile([P, P], F32)
        nc.gpsimd.dma_start(ident_f[:], eye_d)

        W_emb_sb = const.tile([P, 4, EMB], F32)
        nc.gpsimd.dma_start(W_emb_sb[:], W_emb_d.rearrange("(c p) e -> p c e", p=P))
        W_h_sb = const.tile([P, 2, HID], F32)
        nc.gpsimd.dma_start(W_h_sb[:], W_h_d.rearrange("(c p) h -> p c h", p=P))
        b_emb_sb = const.tile([P, 2], F32)
        nc.gpsimd.dma_start(b_emb_sb[:], b_emb_d.rearrange("(c p) -> p c", p=P))
        b_h_sb = const.tile([P, 1], F32)
        nc.gpsimd.dma_start(b_h_sb[:], b_h_d.rearrange("(p o) -> p o", o=1))

        Wk_sb = const.tile([P, P], F32)
        nc.gpsimd.dma_start(Wk_sb[:], Wk_d)
        wk_s_sb = const.tile([P, 1], F32)
        nc.gpsimd.dma_start(wk_s_sb[:], Wqk_d[HID : 2 * HID, :])

        Wv_sb = const.tile([P, P], F32)
        nc.gpsimd.dma_start(Wv_sb[:], Wv_d)
        bv_sb = const.tile([P, 1], F32)
        nc.gpsimd.dma_start(bv_sb[:], bv_d.rearrange("(p o) -> p o", o=1))
        Wa_sb = const.tile([P, P], F32)
        nc.gpsimd.dma_start(Wa_sb[:], Wa_d)
        ba_sb = const.tile([P, 1], F32)
        nc.gpsimd.dma_start(ba_sb[:], ba_d.rearrange("(p o) -> p o", o=1))
        W1_sb = const.tile([P, P], F32)
        nc.gpsimd.dma_start(W1_sb[:], W1_d)
        b1_sb = const.tile([P, 1], F32)
        nc.gpsimd.dma_start(b1_sb[:], b1_d.rearrange("(p o) -> p o", o=1))
        W2_sb = const.tile([P, FEAT], F32)
        nc.gpsimd.dma_start(W2_sb[:], W2_d)
        b2_row = const.tile([1, FEAT], F32)
        nc.gpsimd.dma_start(b2_row[:], b2_d.rearrange("(o f) -> o f", o=1))

        # ---------------- setup folds (fp32) ----------------
        # W_embT blocks: [e-chunk 128, f 512] x2
        W_embT = []
        for ec in range(2):
            t = const.tile([P, FEAT], F32, tag=f"wembT{ec}")
            W_embT.append(t)
            for fc in range(4):
                ps = x_ps.tile([P, FEAT], F32, tag="xps")
                nc.tensor.transpose(
                    ps[:, :P], W_emb_sb[:, fc, ec * P : (ec + 1) * P], ident_f[:]
                )
                nc.vector.tensor_copy(t[:, fc * P : (fc + 1) * P], ps[:, :P])

        # W_fold chunks [f-chunk 128, h] (bf16)
        W_fold = []
        for fc in range(4):
            ps = x_ps.tile([P, FEAT], F32, tag="xps")
            for ec in range(2):
                nc.tensor.matmul(
                    ps[:, :HID],
                    W_embT[ec][:, fc * P : (fc + 1) * P],
                    W_h_sb[:, ec, :],
                    start=(ec == 0),
                    stop=(ec == 1),
                )
            t = const.tile([P, HID], BF16, tag=f"wfold{fc}")
            W_fold.append(t)
            nc.vector.tensor_copy(t[:], ps[:, :HID])

        # b_fold[h] = W_h.T @ b_emb + b_h   -> [128, 1] fp32
        ps = x_ps.tile([P, FEAT], F32, tag="xps")
        for ec in range(2):
            nc.tensor.matmul(
                ps[:, :1],
                W_h_sb[:, ec, :],
                b_emb_sb[:, ec : ec + 1],
                start=(ec == 0),
                stop=(ec == 1),
            )
        b_fold = const.tile([P, 1], F32)
        nc.vector.tensor_add(b_fold[:], ps[:, :1], b_h_sb[:])

        # u = Wk @ wk_s  -> [128, 1]  (needs Wk^T as lhsT)
        ps = x_ps.tile([P, FEAT], F32, tag="xps")
        nc.tensor.transpose(ps[:, :P], Wk_sb[:], ident_f[:])
        WkT = const.tile([P, P], F32)
        nc.vector.tensor_copy(WkT[:], ps[:, :P])
        ps = x_ps.tile([P, FEAT], F32, tag="xps")
        nc.tensor.matmul(ps[:, :1], WkT[:], wk_s_sb[:], start=True, stop=True)
        u_col = const.tile([P, 1], F32)
        nc.vector.tensor_copy(u_col[:], ps[:, :1])
        # U128[h, m] = u[h] for all m  (replicated-u stationary for the
        # broadcast-sk matmul)
        ones_hh = const.tile([P, P], F32)
        nc.vector.memset(ones_hh[:], 1.0)
        U128 = const.tile([P, P], BF16)
        nc.vector.tensor_scalar_mult(U128[:], ones_hh[:], u_col[:])

        # Wva = Wv @ Wa, bva = Wa.T @ bv + ba  (rounds fold: no relu between)
        ps = x_ps.tile([P, FEAT], F32, tag="xps")
        nc.tensor.transpose(ps[:, :P], Wv_sb[:], ident_f[:])
        WvT = const.tile([P, P], F32)
        nc.vector.tensor_copy(WvT[:], ps[:, :P])
        ps = x_ps.tile([P, FEAT], F32, tag="xps")
        nc.tensor.matmul(ps[:, :HID], WvT[:], Wa_sb[:], start=True, stop=True)
        Wva = const.tile([P, P], F32)
        nc.vector.tensor_copy(Wva[:], ps[:, :HID])
        ps = x_ps.tile([P, FEAT], F32, tag="xps")
        nc.tensor.matmul(ps[:, :1], Wa_sb[:], bv_sb[:], start=True, stop=True)
        bva = const.tile([P, 1], F32)
        nc.vector.tensor_add(bva[:], ps[:, :1], ba_sb[:])

        ones_f = const.tile([1, P], F32)
        nc.vector.memset(ones_f[:], 1.0)
        ones_r = const.tile([1, P], F32R)
        nc.vector.tensor_copy(ones_r[:], ones_f[:])

        if v3:
            ident_x = const.tile([P, P], BF16)
            nc.vector.tensor_copy(ident_x[:], ident_f[:])
            tp_dt = BF16
            ld_dt = BF16
        else:
            ident_x = None  # use ident_f bitcast below
            tp_dt = F32R
            ld_dt = F32

        acc = const.tile([P, CW], F32)
        s_row4 = const.tile([1, CW], F32)
        esc_dummy = const.tile([P, FEAT], F32)
        nc.vector.memset(esc_dummy[:], 0.0)

        EV = {"v": nc.vector, "s": nc.scalar, "p": nc.gpsimd}
        s_engine = EV[s_eng]

        rep_ctx = tc.For_i(0, repeat, 1) if repeat > 1 else None
        if rep_ctx is not None:
            rep_ctx.__enter__()
        nc.vector.memset(acc[:], 0.0)
        nc.vector.memset(s_row4[:], 0.0)

        def do_load(li):
            blk = load.tile([P, TPL, FEAT], ld_dt, tag="asblk")
            src = As_d[li * TPL * P : (li + 1) * TPL * P, :].rearrange(
                "(t p) f -> p t f", p=P
            )
            if v3:
                nc.gpsimd.dma_start(blk[:], src)   # SWDGE cast f32->bf16
            else:
                nc.sync.dma_start(blk[:], src)
            return blk

        def step(blk, q, si):
            """Process node-tiles [si*TPS, si*TPS+TPS) = blk[:, q*TPS:(q+1)*TPS]."""
            if upto == "dma":
                junk = work.tile([P, 1], F32, tag="junk")
                eng = nc.vector if si % 2 == 0 else nc.gpsimd
                eng.tensor_copy(junk[:], blk[:, q * TPS, 0:1].bitcast(F32) if v3 else blk[:, q * TPS, 0:1])
                return
            xp = x_ps.tile([P, CW], F32, tag="xps")
            ast = astp.tile([P, 4, CW], BF16, tag="ast")
            for c in range(4):
                tp = tp_ps.tile([P, CW], tp_dt, tag="tpps")
                for t in range(TPS):
                    src = blk[:, q * TPS + t, c * P : (c + 1) * P]
                    if v3:
                        nc.tensor.transpose(
                            tp[:, t * P : (t + 1) * P], src, ident_x[:]
                        )
                    else:
                        nc.tensor.transpose(
                            tp[:, t * P : (t + 1) * P],
                            src.bitcast(F32R),
                            ident_f[:].bitcast(F32R),
                        )
                if upto == "tp":
                    continue
                tp_src = tp[:] if v3 else tp[:].bitcast(F32)
                eng = EV[evict_eng[c]]
                if evict_eng[c] == "s":
                    eng.copy(ast[:, c, :], tp_src)
                else:
                    eng.tensor_copy(ast[:, c, :], tp_src)
                if upto == "evict":
                    continue
                nc.tensor.matmul(
                    xp[:], W_fold[c][:], ast[:, c, :],
                    start=(c == 0), stop=(c == 3),
                )
            if upto in ("tp", "evict", "mm"):
                return
            xslice = xsl.tile([P, CW], BF16, tag="xsl")
            nc.scalar.activation(xslice[:], xp[:], AF.Relu, bias=b_fold[:])
            if upto == "relu":
                return

            # broadcast sk: skb[m, col] = sum_h u[h] xT[h, col]  (same for all m)
            skb = sk_ps.tile([P, CW], F32, tag="skps")
            nc.tensor.matmul(skb[:], U128[:], xslice[:], start=True, stop=True)
            if upto == "skb":
                return
            # unnormalized attention: scores are O(0.2) so exp needs no
            # max-subtraction for stability
            e_wb = work.tile([P, CW], F32, tag="ewb")
            nc.scalar.activation(e_wb[:], skb[:], AF.Exp)
            if upto == "exp":
                return
            s_engine.tensor_add(s_row4[:], s_row4[:], e_wb[0:1, :])
            tmp = work.tile([P, CW], F32, tag="aggtmp")
            nc.vector.tensor_mul(tmp[:], xslice[:], e_wb[:])
            nc.vector.tensor_add(acc[:], acc[:], tmp[:])

        for li in range(NLOADS):
            blk = do_load(li)
            for q in range(TPL // TPS):
                step(blk, q, li * (TPL // TPS) + q)

        if upto != "full":
            nc.sync.dma_start(out_d, esc_dummy[:])
        else:
            # ---------------- normalization: acc / sum_j exp(sk) ----------------
            # fold (t,b) columns: acc[:, b] = sum_t acc[:, t*128+b]
            nc.vector.tensor_add(acc[:, :256], acc[:, :256], acc[:, 256:512])
            nc.vector.tensor_add(acc[:, :128], acc[:, :128], acc[:, 128:256])
            nc.vector.tensor_add(s_row4[:, :256], s_row4[:, :256], s_row4[:, 256:512])
            nc.vector.tensor_add(s_row4[:, :128], s_row4[:, :128], s_row4[:, 128:256])
            rcp_f = const.tile([1, P], F32)
            nc.vector.reciprocal(rcp_f[:], s_row4[:, :128])
            rcp_r = const.tile([1, P], F32R)
            nc.vector.tensor_copy(rcp_r[:], rcp_f[:])
            rb = sk_ps.tile([P, CW], F32, tag="skps")
            nc.tensor.matmul(rb[:, :P], ones_r[:], rcp_r[:], start=True, stop=True)
            xaggT_t = const.tile([P, P], F32)
            nc.vector.tensor_mul(xaggT_t[:], acc[:, :P], rb[:, :P])
            xaggT = xaggT_t[:]

            # ---------------- rounds + readout ----------------
            def dense(inp, W_sb, bias, relu, name, dt_out=F32):
                ps2 = x_ps.tile([P, CW], F32, tag="xps")
                nc.tensor.matmul(ps2[:, :HID], W_sb[:], inp, start=True, stop=True)
                o = const.tile([P, P], dt_out, tag=name)
                nc.scalar.activation(
                    o[:], ps2[:, :HID], AF.Relu if relu else AF.Identity, bias=bias[:]
                )
                return o[:]

            cur = xaggT
            for r in range(3):
                cur = dense(cur, Wva[:], bva, True, f"y{r}")

            rT = dense(cur, W1_sb, b1_sb, True, "rT", dt_out=F32R)
            # logits [b, f] = rT.T @ W2 + b2  (f32r, PSUM-accumulated bias)
            W2_r = const.tile([P, FEAT], F32R)
            nc.vector.tensor_copy(W2_r[:], W2_sb[:])
            b2_row_r = const.tile([1, FEAT], F32R)
            nc.vector.tensor_copy(b2_row_r[:], b2_row[:])
            lps = x_ps.tile([P, FEAT], F32, tag="xps")
            nc.tensor.matmul(lps[:], rT, W2_r[:], start=True, stop=False)
            nc.tensor.matmul(lps[:], ones_r[:], b2_row_r[:], start=False, stop=True)
            # log_softmax along f; logits are O(0.3) so no max subtraction needed
            esc = const.tile([P, FEAT], F32)
            s2 = const.tile([P, 1], F32)
            nc.scalar.activation(esc[:], lps[:], AF.Exp, accum_out=s2[:])
            lns = const.tile([P, 1], F32)
            nc.scalar.activation(lns[:], s2[:], AF.Ln)
            final = const.tile([P, FEAT], F32)
            nc.vector.tensor_scalar_sub(final[:], lps[:], lns[:])
            nc.sync.dma_start(out_d, final[:])

        if rep_ctx is not None:
            rep_ctx.__exit__(None, None, None)

    nc.compile()
    return nc


_NC = None


def _get_nc():
    global _NC
    if _NC is None:
        _NC = build()
    return _NC


def kernel(**inputs):
    inp = {k: np.asarray(v, dtype=np.float32) for k, v in inputs.items()}
    As = inp["As"]  # [128, 1024, 512]
    eye = np.eye(P, dtype=np.float32)
    names = ["W_emb", "b_emb", "W_h", "b_h", "Wk", "Wqk", "Wv", "bv",
             "Wa", "ba", "W1", "b1", "W2", "b2"]
    in_maps = []
    for c in range(NCORES):
        shard = np.ascontiguousarray(
            As[:, c * BLOC : (c + 1) * BLOC, :]
        ).reshape(ROWS, FEAT)
        m = {"As": shard, "eye": eye}
        for n in names:
            m[n] = inp[n]
        in_maps.append(m)
    res = run_bass_kernel_spmd(_get_nc(), in_maps, list(range(NCORES))).results
    return np.concatenate([res[c]["out"] for c in range(NCORES)], axis=0)


# revision 4
# speedup vs baseline: 1.3877x; 1.3877x over previous
"""Trainium2 Bass kernel for the GNN message-passing draft problem.

Math notes (exact simplifications of the reference):
- softmax over key nodes j makes scores' sq/bqk terms cancel
  (shift invariance), so w[i,j,b] = softmax_j(sk[j,b]) independent of i.
- Therefore after round 1 the node state is constant across nodes, and
  rounds 2/3 collapse to per-batch MLPs:  x <- relu((x@Wv+bv)@Wa+ba).
- Round 1 aggregation commutes with Wv:  aggre = (sum_j w[j,b] x_j) @ Wv + bv.
- (As@W_emb + b_emb)@W_h + b_h == As@(W_emb@W_h) + (b_emb@W_h + b_h).
- Wq, bq, bk, bqk never affect the output.

Per core (8 cores, data-parallel over batch): As shard [N=128 nodes,
B_loc=128, F=512] flattened to rows (j,b) j-major = [16384, 512].
Stage 1 streams As, transposes 128x128 blocks on PE, and runs matmuls
against the folded weight to produce xT [h=128, cols=(j,b)].

Performance structure (vs the earlier baseline):
- sk is computed with a replicated-u stationary so one matmul yields sk
  broadcast across all 128 partitions; exp of that directly gives the
  broadcast attention weights (no separate ones-outer-product matmul).
- softmax denominator accumulated as one [1,512] add per step instead of
  4 gpsimd adds.
- transposes run in f32r (1.5 cyc/row) or bf16 (1 cyc/row, via SWDGE
  cast-DMA loads); evictions are spread across DVE/ACT/Pool.
- As is loaded in 2 MiB transfers (8 node-tiles), 2 compute steps each.
"""

import sys

sys.path.insert(0, "/opt/trn_rl_repo")

from contextlib import ExitStack

import numpy as np

import concourse.bass as bass
import concourse.tile as tile
from concourse import bacc, mybir
from concourse.bass_utils import run_bass_kernel_spmd

F32 = mybir.dt.float32
F32R = mybir.dt.float32r
BF16 = mybir.dt.bfloat16
AF = mybir.ActivationFunctionType
ALU = mybir.AluOpType

N_NODES, BATCH, FEAT, EMB, HID = 128, 1024, 512, 256, 128
NCORES = 8
BLOC = BATCH // NCORES          # 128 batch elements per core
ROWS = N_NODES * BLOC           # 16384 rows per core
P = 128
TPS = 4                         # node-tiles per compute step
TPL = 8                         # node-tiles per DMA transfer (2 MiB f32)
NSTEPS = N_NODES // TPS         # 32 compute steps
NLOADS = N_NODES // TPL         # 16 DMA transfers
CW = TPS * P                    # 512 columns per compute step


def build(repeat=1, upto="full", variant="bf16", evict_eng="vvvv",
          s_eng="p", load_bufs=4):
    """variant: 'bf16' = SWDGE cast-DMA loads + bf16 transposes;
    'f32r' = plain HWDGE f32 loads + f32r transposes."""
    nc = bacc.Bacc(None, target_bir_lowering=False, debug=False)
    v3 = variant == "bf16"

    dI = lambda name, shape: nc.dram_tensor(name, shape, F32, kind="ExternalInput").ap()
    As_d = dI("As", [ROWS, FEAT])
    W_emb_d = dI("W_emb", [FEAT, EMB])
    b_emb_d = dI("b_emb", [EMB])
    W_h_d = dI("W_h", [EMB, HID])
    b_h_d = dI("b_h", [HID])
    Wk_d = dI("Wk", [HID, HID])
    Wqk_d = dI("Wqk", [2 * HID, 1])
    Wv_d = dI("Wv", [HID, HID])
    bv_d = dI("bv", [HID])
    Wa_d = dI("Wa", [HID, HID])
    ba_d = dI("ba", [HID])
    W1_d = dI("W1", [HID, HID])
    b1_d = dI("b1", [HID])
    W2_d = dI("W2", [HID, FEAT])
    b2_d = dI("b2", [FEAT])
    eye_d = dI("eye", [P, P])
    out_d = nc.dram_tensor("out", [BLOC, FEAT], F32, kind="ExternalOutput").ap()

    with tile.TileContext(nc) as tc, ExitStack() as ctx:
        const = ctx.enter_context(tc.tile_pool(name="const", bufs=1))
        work = ctx.enter_context(tc.tile_pool(name="work", bufs=3))
        load = ctx.enter_context(tc.tile_pool(name="load", bufs=load_bufs))
        astp = ctx.enter_context(tc.tile_pool(name="astp", bufs=3))
        xsl = ctx.enter_context(tc.tile_pool(name="xsl", bufs=3))
        tp_ps = ctx.enter_context(tc.tile_pool(name="tp_ps", bufs=3, space="PSUM"))
        x_ps = ctx.enter_context(tc.tile_pool(name="x_ps", bufs=2, space="PSUM"))
        sk_ps = ctx.enter_context(tc.tile_pool(name="sk_ps", bufs=2, space="PSUM"))

        # ---------------- constants / weights ----------------
        ident_f = const.tile([P, P], F32)
        nc.gpsimd.dma_start(ident_f[:], eye_d)

        W_emb_sb = const.tile([P, 4, EMB], F32)
        nc.gpsimd.dma_start(W_emb_sb[:], W_emb_d.rearrange("(c p) e -> p c e", p=P))
        W_h_sb = const.tile([P, 2, HID], F32)
        nc.gpsimd.dma_start(W_h_sb[:], W_h_d.rearrange("(c p) h -> p c h", p=P))
        b_emb_sb = const.tile([P, 2], F32)
        nc.gpsimd.dma_start(b_emb_sb[:], b_emb_d.rearrange("(c p) -> p c", p=P))
        b_h_sb = const.tile([P, 1], F32)
        nc.gpsimd.dma_start(b_h_sb[:], b_h_d.rearrange("(p o) -> p o", o=1))

        Wk_sb = const.tile([P, P], F32)
        nc.gpsimd.dma_start(Wk_sb[:], Wk_d)
        wk_s_sb = const.tile([P, 1], F32)
        nc.gpsimd.dma_start(wk_s_sb[:], Wqk_d[HID : 2 * HID, :])

        Wv_sb = const.tile([P, P], F32)
        nc.gpsimd.dma_start(Wv_sb[:], Wv_d)
        bv_sb = const.tile([P, 1], F32)
        nc.gpsimd.dma_start(bv_sb[:], bv_d.rearrange("(p o) -> p o", o=1))
        Wa_sb = const.tile([P, P], F32)
        nc.gpsimd.dma_start(Wa_sb[:], Wa_d)
        ba_sb = const.tile([P, 1], F32)
        nc.gpsimd.dma_start(ba_sb[:], ba_d.rearrange("(p o) -> p o", o=1))
        W1_sb = const.tile([P, P], F32)
        nc.gpsimd.dma_start(W1_sb[:], W1_d)
        b1_sb = const.tile([P, 1], F32)
        nc.gpsimd.dma_start(b1_sb[:], b1_d.rearrange("(p o) -> p o", o=1))
        W2_sb = const.tile([P, FEAT], F32)
        nc.gpsimd.dma_start(W2_sb[:], W2_d)
        b2_row = const.tile([1, FEAT], F32)
        nc.gpsimd.dma_start(b2_row[:], b2_d.rearrange("(o f) -> o f", o=1))

        # ---------------- setup folds (fp32) ----------------
        # W_embT blocks: [e-chunk 128, f 512] x2
        W_embT = []
        for ec in range(2):
            t = const.tile([P, FEAT], F32, tag=f"wembT{ec}")
            W_embT.append(t)
            for fc in range(4):
                ps = x_ps.tile([P, FEAT], F32, tag="xps")
                nc.tensor.transpose(
                    ps[:, :P], W_emb_sb[:, fc, ec * P : (ec + 1) * P], ident_f[:]
                )
                nc.vector.tensor_copy(t[:, fc * P : (fc + 1) * P], ps[:, :P])

        # W_fold chunks [f-chunk 128, h] (bf16)
        W_fold = []
        for fc in range(4):
            ps = x_ps.tile([P, FEAT], F32, tag="xps")
            for ec in range(2):
                nc.tensor.matmul(
                    ps[:, :HID],
                    W_embT[ec][:, fc * P : (fc + 1) * P],
                    W_h_sb[:, ec, :],
                    start=(ec == 0),
                    stop=(ec == 1),
                )
            t = const.tile([P, HID], BF16, tag=f"wfold{fc}")
            W_fold.append(t)
            nc.vector.tensor_copy(t[:], ps[:, :HID])

        # b_fold[h] = W_h.T @ b_emb + b_h   -> [128, 1] fp32
        ps = x_ps.tile([P, FEAT], F32, tag="xps")
        for ec in range(2):
            nc.tensor.matmul(
                ps[:, :1],
                W_h_sb[:, ec, :],
                b_emb_sb[:, ec : ec + 1],
                start=(ec == 0),
                stop=(ec == 1),
            )
        b_fold = const.tile([P, 1], F32)
        nc.vector.tensor_add(b_fold[:], ps[:, :1], b_h_sb[:])

        # u = Wk @ wk_s  -> [128, 1]  (needs Wk^T as lhsT)
        ps = x_ps.tile([P, FEAT], F32, tag="xps")
        nc.tensor.transpose(ps[:, :P], Wk_sb[:], ident_f[:])
        WkT = const.tile([P, P], F32)
        nc.vector.tensor_copy(WkT[:], ps[:, :P])
        ps = x_ps.tile([P, FEAT], F32, tag="xps")
        nc.tensor.matmul(ps[:, :1], WkT[:], wk_s_sb[:], start=True, stop=True)
        u_col = const.tile([P, 1], F32)
        nc.vector.tensor_copy(u_col[:], ps[:, :1])
        # U128[h, m] = u[h] for all m  (replicated-u stationary for the
        # broadcast-sk matmul)
        ones_hh = const.tile([P, P], F32)
        nc.vector.memset(ones_hh[:], 1.0)
        U128 = const.tile([P, P], BF16)
        nc.vector.tensor_scalar_mul(U128[:], ones_hh[:], u_col[:])

        # Wva = Wv @ Wa, bva = Wa.T @ bv + ba  (rounds fold: no relu between)
        ps = x_ps.tile([P, FEAT], F32, tag="xps")
        nc.tensor.transpose(ps[:, :P], Wv_sb[:], ident_f[:])
        WvT = const.tile([P, P], F32)
        nc.vector.tensor_copy(WvT[:], ps[:, :P])
        ps = x_ps.tile([P, FEAT], F32, tag="xps")
        nc.tensor.matmul(ps[:, :HID], WvT[:], Wa_sb[:], start=True, stop=True)
        Wva = const.tile([P, P], F32)
        nc.vector.tensor_copy(Wva[:], ps[:, :HID])
        ps = x_ps.tile([P, FEAT], F32, tag="xps")
        nc.tensor.matmul(ps[:, :1], Wa_sb[:], bv_sb[:], start=True, stop=True)
        bva = const.tile([P, 1], F32)
        nc.vector.tensor_add(bva[:], ps[:, :1], ba_sb[:])

        ones_f = const.tile([1, P], F32)
        nc.vector.memset(ones_f[:], 1.0)
        ones_r = const.tile([1, P], F32R)
        nc.vector.tensor_copy(ones_r[:], ones_f[:])

        if v3:
            ident_x = const.tile([P, P], BF16)
            nc.vector.tensor_copy(ident_x[:], ident_f[:])
            tp_dt = BF16
            ld_dt = BF16
        else:
            ident_x = None  # use ident_f bitcast below
            tp_dt = F32R
            ld_dt = F32

        acc = const.tile([P, CW], F32)
        s_row4 = const.tile([1, CW], F32)
        esc_dummy = const.tile([P, FEAT], F32)
        nc.vector.memset(esc_dummy[:], 0.0)

        EV = {"v": nc.vector, "s": nc.scalar, "p": nc.gpsimd}
        s_engine = EV[s_eng]

        rep_ctx = tc.For_i(0, repeat, 1) if repeat > 1 else None
        if rep_ctx is not None:
            rep_ctx.__enter__()
        nc.vector.memset(acc[:], 0.0)
        nc.vector.memset(s_row4[:], 0.0)

        def do_load(li):
            blk = load.tile([P, TPL, FEAT], ld_dt, tag="asblk")
            src = As_d[li * TPL * P : (li + 1) * TPL * P, :].rearrange(
                "(t p) f -> p t f", p=P
            )
            if v3:
                nc.gpsimd.dma_start(blk[:], src)   # SWDGE cast f32->bf16
            else:
                nc.sync.dma_start(blk[:], src)
            return blk

        # Software pipelining: the attention chain of step s-1 (skb -> exp ->
        # s-add/mul/add) is emitted at the head of step s, so skb(s-1) sits at
        # the top of the PE FIFO (ready: relu(s-1) just finished) instead of
        # making step s's transposes wait behind a PE instruction whose input
        # isn't ready yet.
        pend = [None]

        def flush_pend():
            xslice = pend[0]
            if xslice is None:
                return
            pend[0] = None
            # broadcast sk: skb[m, col] = sum_h u[h] xT[h, col]  (same for all m)
            skb = sk_ps.tile([P, CW], F32, tag="skps")
            nc.tensor.matmul(skb[:], U128[:], xslice, start=True, stop=True)
            if upto == "skb":
                return
            # unnormalized attention: scores are O(0.2) so exp needs no
            # max-subtraction for stability
            e_wb = work.tile([P, CW], F32, tag="ewb")
            nc.scalar.activation(e_wb[:], skb[:], AF.Exp)
            if upto == "exp":
                return
            s_engine.tensor_add(s_row4[:], s_row4[:], e_wb[0:1, :])
            tmp = work.tile([P, CW], F32, tag="aggtmp")
            nc.vector.tensor_mul(tmp[:], xslice, e_wb[:])
            nc.vector.tensor_add(acc[:], acc[:], tmp[:])

        def step(blk, q, si):
            """Process node-tiles blk[:, q*TPS:(q+1)*TPS]."""
            if upto == "dma":
                junk = work.tile([P, 1], F32, tag="junk")
                eng = nc.vector if si % 2 == 0 else nc.gpsimd
                eng.tensor_copy(junk[:], blk[:, q * TPS, 0:1])
                return
            flush_pend()
            xp = x_ps.tile([P, CW], F32, tag="xps")
            ast = astp.tile([P, 4, CW], BF16, tag="ast")
            for c in range(4):
                tp = tp_ps.tile([P, CW], tp_dt, tag="tpps")
                for t in range(TPS):
                    src = blk[:, q * TPS + t, c * P : (c + 1) * P]
                    if v3:
                        nc.tensor.transpose(
                            tp[:, t * P : (t + 1) * P], src, ident_x[:]
                        )
                    else:
                        nc.tensor.transpose(
                            tp[:, t * P : (t + 1) * P],
                            src.bitcast(F32R),
                            ident_f[:].bitcast(F32R),
                        )
                if upto == "tp":
                    continue
                tp_src = tp[:] if v3 else tp[:].bitcast(F32)
                eng = EV[evict_eng[c]]
                if evict_eng[c] == "s":
                    eng.copy(ast[:, c, :], tp_src)
                else:
                    eng.tensor_copy(ast[:, c, :], tp_src)
                if upto == "evict":
                    continue
                nc.tensor.matmul(
                    xp[:], W_fold[c][:], ast[:, c, :],
                    start=(c == 0), stop=(c == 3),
                )
            if upto in ("tp", "evict", "mm"):
                return
            xslice = xsl.tile([P, CW], BF16, tag="xsl")
            nc.scalar.activation(xslice[:], xp[:], AF.Relu, bias=b_fold[:])
            if upto == "relu":
                return
            pend[0] = xslice[:]

        for li in range(NLOADS):
            blk = do_load(li)
            for q in range(TPL // TPS):
                step(blk, q, li * (TPL // TPS) + q)
        flush_pend()

        if upto != "full":
            nc.sync.dma_start(out_d, esc_dummy[:])
        else:
            # ---------------- normalization: acc / sum_j exp(sk) ----------------
            # fold (t,b) columns: acc[:, b] = sum_t acc[:, t*128+b]
            nc.vector.tensor_add(acc[:, :256], acc[:, :256], acc[:, 256:512])
            nc.vector.tensor_add(acc[:, :128], acc[:, :128], acc[:, 128:256])
            nc.vector.tensor_add(s_row4[:, :256], s_row4[:, :256], s_row4[:, 256:512])
            nc.vector.tensor_add(s_row4[:, :128], s_row4[:, :128], s_row4[:, 128:256])
            rcp_f = const.tile([1, P], F32)
            nc.vector.reciprocal(rcp_f[:], s_row4[:, :128])
            rcp_r = const.tile([1, P], F32R)
            nc.vector.tensor_copy(rcp_r[:], rcp_f[:])
            rb = sk_ps.tile([P, CW], F32, tag="skps")
            nc.tensor.matmul(rb[:, :P], ones_r[:], rcp_r[:], start=True, stop=True)
            xaggT_t = const.tile([P, P], F32)
            nc.vector.tensor_mul(xaggT_t[:], acc[:, :P], rb[:, :P])
            xaggT = xaggT_t[:]

            # ---------------- rounds + readout ----------------
            def dense(inp, W_sb, bias, relu, name, dt_out=F32):
                ps2 = x_ps.tile([P, CW], F32, tag="xps")
                nc.tensor.matmul(ps2[:, :HID], W_sb[:], inp, start=True, stop=True)
                o = const.tile([P, P], dt_out, tag=name)
                nc.scalar.activation(
                    o[:], ps2[:, :HID], AF.Relu if relu else AF.Identity, bias=bias[:]
                )
                return o[:]

            cur = xaggT
            for r in range(3):
                cur = dense(cur, Wva[:], bva, True, f"y{r}")

            rT = dense(cur, W1_sb, b1_sb, True, "rT", dt_out=F32R)
            # logits [b, f] = rT.T @ W2 + b2  (f32r, PSUM-accumulated bias)
            W2_r = const.tile([P, FEAT], F32R)
            nc.vector.tensor_copy(W2_r[:], W2_sb[:])
            b2_row_r = const.tile([1, FEAT], F32R)
            nc.vector.tensor_copy(b2_row_r[:], b2_row[:])
            lps = x_ps.tile([P, FEAT], F32, tag="xps")
            nc.tensor.matmul(lps[:], rT, W2_r[:], start=True, stop=False)
            nc.tensor.matmul(lps[:], ones_r[:], b2_row_r[:], start=False, stop=True)
            # log_softmax along f; logits are O(0.3) so no max subtraction needed
            esc = const.tile([P, FEAT], F32)
            s2 = const.tile([P, 1], F32)
            nc.scalar.activation(esc[:], lps[:], AF.Exp, accum_out=s2[:])
            lns = const.tile([P, 1], F32)
            nc.scalar.activation(lns[:], s2[:], AF.Ln)
            final = const.tile([P, FEAT], F32)
            nc.vector.tensor_scalar_sub(final[:], lps[:], lns[:])
            nc.sync.dma_start(out_d, final[:])

        if rep_ctx is not None:
            rep_ctx.__exit__(None, None, None)

    nc.compile()
    return nc


_NC = None


def _get_nc():
    global _NC
    if _NC is None:
        _NC = build()
    return _NC


def kernel(**inputs):
    inp = {k: np.asarray(v, dtype=np.float32) for k, v in inputs.items()}
    As = inp["As"]  # [128, 1024, 512]
    eye = np.eye(P, dtype=np.float32)
    names = ["W_emb", "b_emb", "W_h", "b_h", "Wk", "Wqk", "Wv", "bv",
             "Wa", "ba", "W1", "b1", "W2", "b2"]
    in_maps = []
    for c in range(NCORES):
        shard = np.ascontiguousarray(
            As[:, c * BLOC : (c + 1) * BLOC, :]
        ).reshape(ROWS, FEAT)
        m = {"As": shard, "eye": eye}
        for n in names:
            m[n] = inp[n]
        in_maps.append(m)
    res = run_bass_kernel_spmd(_get_nc(), in_maps, list(range(NCORES))).results
    return np.concatenate([res[c]["out"] for c in range(NCORES)], axis=0)
